# revision 4
# baseline (speedup 1.0000x reference)
"""GPT-2 attention block (B=4, S=1024, D=1024, H=16) on 8 TRN2 NeuronCores.

Tensor-parallel over heads: core i holds heads 2i, 2i+1. qkv is computed
with per-core weight columns; attention scores are built directly in
transposed layout P^T[k, q] so they feed the AV matmul as the moving
operand with no transposes; the softmax denominator rides along the AV
matmul as an appended ones-column block of the stationary operand. An
AllToAll converts head-sharding to token-sharding so c_proj needs no
reduction: each core emits a disjoint [D, 512]-token output shard.
"""

import numpy as np
import ml_dtypes

import concourse.bass as bass
import concourse.mybir as mybir
import concourse.tile as tile
from concourse import bacc
from concourse.bass_utils import run_bass_kernel_spmd

B, S, D, H = 4, 1024, 1024, 16
HD = D // H  # 64
NT = B * S  # 4096 tokens
N_CORES = 8
CORE_IDS = list(range(N_CORES))
NTC = NT // 512  # 8 token chunks of 512
BF16 = mybir.dt.bfloat16
F32 = mybir.dt.float32
AF = mybir.ActivationFunctionType

_CACHE = {}


def build_nc():
    nc = bacc.Bacc("TRN2", target_bir_lowering=False, debug=False, num_devices=N_CORES)

    xt_d = nc.dram_tensor("xt", [D, NT], BF16, kind="ExternalInput")
    wqkv_d = nc.dram_tensor("wqkv", [D, 384], BF16, kind="ExternalInput")
    bqk_d = nc.dram_tensor("bqk", [2, 128, 1], F32, kind="ExternalInput")
    bv_d = nc.dram_tensor("bv", [1, 128], BF16, kind="ExternalInput")
    wp_d = nc.dram_tensor("wp", [D, D], BF16, kind="ExternalInput")
    tri_d = nc.dram_tensor("tri", [128, 128], F32, kind="ExternalInput")
    out_d = nc.dram_tensor("out", [D, 512], F32, kind="ExternalOutput")

    with tile.TileContext(nc) as tc:
        with (
            tc.tile_pool(name="persist", bufs=1) as pp,
            tc.tile_pool(name="xin", bufs=3) as xp,
            tc.tile_pool(name="ptp", bufs=6) as ptp,
            tc.tile_pool(name="work", bufs=2) as wk,
            tc.tile_pool(name="ps_big", bufs=2, space="PSUM") as ps_big,
            tc.tile_pool(name="ps_pt", bufs=4, space="PSUM") as ps_pt,
            tc.tile_pool(name="ps_at", bufs=1, space="PSUM") as ps_at,
            tc.tile_pool(name="dram", bufs=1, space="DRAM") as dp,
        ):
            # ---- persistent weights / constants ----
            wqkv = []
            for k in range(8):
                t = pp.tile([128, 384], BF16, tag=f"wqkv{k}")
                nc.sync.dma_start(t[:], wqkv_d[128 * k : 128 * (k + 1), :])
                wqkv.append(t)
            wp_sb = []
            for k in range(8):
                t = pp.tile([128, D], BF16, tag=f"wp{k}")
                nc.sync.dma_start(t[:], wp_d[128 * k : 128 * (k + 1), :])
                wp_sb.append(t)
            tri = pp.tile([128, 128], F32, tag="tri")
            nc.sync.dma_start(tri[:], tri_d[:])
            bq = pp.tile([128, 1], F32, tag="bq")
            nc.sync.dma_start(bq[:], bqk_d[0])
            bk = pp.tile([128, 1], F32, tag="bk")
            nc.sync.dma_start(bk[:], bqk_d[1])
            bv = pp.tile([1, 128], BF16, tag="bv")
            nc.sync.dma_start(bv[:], bv_d[:])
            ones1 = pp.tile([1, 128], BF16, tag="ones1")
            nc.gpsimd.memset(ones1[:], 1.0)

            # ---- phase 1: qkv ----
            qt, kt = [], []
            vaug = {}
            for t in range(NTC):
                xts = []
                for k in range(8):
                    xtile = xp.tile([128, 512], BF16, tag=f"x{k}")
                    nc.sync.dma_start(
                        xtile[:], xt_d[128 * k : 128 * (k + 1), 512 * t : 512 * (t + 1)]
                    )
                    xts.append(xtile)
                # qT, kT (transposed layout: partitions = qkv cols of 2 heads)
                for m, (store, bias) in enumerate(((qt, bq), (kt, bk))):
                    ps = ps_big.tile([128, 512], F32, tag="mm")
                    for k in range(8):
                        nc.tensor.matmul(
                            ps[:],
                            wqkv[k][:, 128 * m : 128 * (m + 1)],
                            xts[k][:],
                            start=(k == 0),
                            stop=(k == 7),
                        )
                    sb = pp.tile([128, 512], BF16, tag=f"qk{m}_{t}")
                    nc.scalar.activation(sb[:], ps[:], AF.Identity, bias=bias[:])
                    store.append(sb)
                # v natural layout: partitions = tokens (4 sub-chunks of 128)
                vps = ps_big.tile([128, 512], F32, tag="mm")
                for i in range(4):
                    seg = vps[:, 128 * i : 128 * (i + 1)]
                    for k in range(8):
                        nc.tensor.matmul(
                            seg,
                            xts[k][:, 128 * i : 128 * (i + 1)],
                            wqkv[k][:, 256:384],
                            start=(k == 0),
                            stop=False,
                        )
                    nc.tensor.matmul(
                        seg, ones1[:], bv[:], start=False, stop=True,
                    )
                for i in range(4):
                    for h in range(2):
                        va = pp.tile([128, 128], BF16, tag=f"va{t}_{i}_{h}")
                        nc.vector.tensor_copy(
                            va[:, 0:64], vps[:, 128 * i + 64 * h : 128 * i + 64 * h + 64]
                        )
                        nc.gpsimd.memset(va[:, 64:128], 1.0)
                        vaug[(t, i, h)] = va

            # ---- phase 2: attention ----
            at_sb = []
            for b in range(B):
                aT = pp.tile([128, 1024], BF16, tag=f"aT{b}")
                at_sb.append(aT)
                for s in range(2):
                    tcq = 2 * b + s
                    last = 4 * s + 3
                    at_ps = [
                        ps_at.tile([128, 512], F32, tag=f"at{h}", name=f"at{h}")
                        for h in range(2)
                    ]
                    for kc in range(last + 1):
                        off = max(0, kc * 128 - s * 512)
                        width = 512 - off
                        tck = 2 * b + kc // 4
                        kcol = (kc % 4) * 128
                        dq = kc * 128 - s * 512  # diag col in span coords
                        for h in range(2):
                            pt_ps = ps_pt.tile([128, 512], F32, tag="pt")
                            nc.tensor.matmul(
                                pt_ps[:, 0:width],
                                kt[tck][64 * h : 64 * h + 64, kcol : kcol + 128],
                                qt[tcq][64 * h : 64 * h + 64, off:512],
                                start=True,
                                stop=True,
                            )
                            if dq >= 0:
                                dcol = dq - off
                                nc.vector.tensor_add(
                                    pt_ps[:, dcol : dcol + 128],
                                    pt_ps[:, dcol : dcol + 128],
                                    tri[:],
                                )
                            pt_sb = ptp.tile([128, 512], BF16, tag="pt")
                            nc.scalar.activation(
                                pt_sb[:, 0:width], pt_ps[:, 0:width], AF.Exp
                            )
                            nc.tensor.matmul(
                                at_ps[h][:, off:512],
                                vaug[(tck, kc % 4, h)][:],
                                pt_sb[:, 0:width],
                                start=(kc == 0),
                                stop=(kc == last),
                            )
                    for h in range(2):
                        rec = wk.tile([64, 512], F32, tag=f"rec{h}")
                        nc.vector.reciprocal(rec[:], at_ps[h][64:128, :])
                        nc.vector.tensor_mul(
                            aT[64 * h : 64 * h + 64, 512 * s : 512 * (s + 1)],
                            at_ps[h][0:64, :],
                            rec[:],
                        )

            # ---- phase 3: AllToAll (head-shard -> token-shard) ----
            a2a_in = dp.tile([1024, 512], BF16)
            a2a_out = dp.tile([1024, 512], BF16)
            for j in range(8):
                nc.sync.dma_start(
                    a2a_in[128 * j : 128 * (j + 1), :],
                    at_sb[j // 2][:, 512 * (j % 2) : 512 * (j % 2 + 1)],
                )
            nc.gpsimd.collective_compute(
                "AllToAll",
                mybir.AluOpType.bypass,
                replica_groups=[CORE_IDS],
                ins=[a2a_in.opt()],
                outs=[a2a_out.opt()],
            )

            # ---- phase 4: c_proj on own token shard ----
            ae = []
            for k in range(8):
                t = pp.tile([128, 512], BF16, tag=f"ae{k}")
                nc.sync.dma_start(t[:], a2a_out[128 * k : 128 * (k + 1), :])
                ae.append(t)
            for m in range(8):
                ps = ps_big.tile([128, 512], F32, tag="mm")
                for k in range(8):
                    nc.tensor.matmul(
                        ps[:],
                        wp_sb[k][:, 128 * m : 128 * (m + 1)],
                        ae[k][:],
                        start=(k == 0),
                        stop=(k == 7),
                    )
                osb = wk.tile([128, 512], F32, tag="osb")
                nc.scalar.activation(osb[:], ps[:], AF.Copy)
                nc.sync.dma_start(out_d[128 * m : 128 * (m + 1), :], osb[:])

    nc.compile()
    return nc


def _prep_inputs(x, w_attn, b_attn, w_proj):
    bf = ml_dtypes.bfloat16
    xt = np.ascontiguousarray(x.reshape(NT, D).T).astype(bf)
    scale = 1.0 / np.sqrt(np.float32(HD))
    wp = w_proj.astype(bf)
    tri = np.where(
        np.arange(128)[:, None] <= np.arange(128)[None, :], 0.0, -10000.0
    ).astype(np.float32)
    in_maps = []
    for i in range(N_CORES):
        c = 128 * i
        wq = (w_attn[:, c : c + 128] * scale).astype(bf)
        wkk = w_attn[:, D + c : D + c + 128].astype(bf)
        wv = w_attn[:, 2 * D + c : 2 * D + c + 128].astype(bf)
        wqkv = np.concatenate([wq, wkk, wv], axis=1)
        bqk = np.stack(
            [
                (b_attn[c : c + 128] * scale).astype(np.float32),
                b_attn[D + c : D + c + 128].astype(np.float32),
            ]
        ).reshape(2, 128, 1)
        bv = b_attn[2 * D + c : 2 * D + c + 128].astype(bf).reshape(1, 128)
        in_maps.append(
            {"xt": xt, "wqkv": wqkv, "bqk": bqk, "bv": bv, "wp": wp, "tri": tri}
        )
    return in_maps


def run_on_hw(in_maps, trace=False, **kw):
    if "nc" not in _CACHE:
        _CACHE["nc"] = build_nc()
    return run_bass_kernel_spmd(_CACHE["nc"], in_maps, CORE_IDS, trace=trace, **kw)


def assemble_output(results, b_proj):
    outT = np.concatenate([results[j]["out"] for j in range(N_CORES)], axis=1)
    return (outT.T + b_proj[None, :].astype(np.float32)).reshape(B, S, D)


def kernel(x, w_attn, b_attn, w_proj, b_proj):
    in_maps = _prep_inputs(
        np.asarray(x, dtype=np.float32),
        np.asarray(w_attn, dtype=np.float32),
        np.asarray(b_attn, dtype=np.float32),
        np.asarray(w_proj, dtype=np.float32),
    )
    res = run_on_hw(in_maps)
    return assemble_output(res.results, np.asarray(b_proj, dtype=np.float32))
